# revision 9
# baseline (speedup 1.0000x reference)
"""Dense+Vanilla Mixture Synthesizer attention kernel for 8 Trainium2 NeuronCores.

Reference computation (per batch b):
    dense  = relu(query @ w1 + b1) @ w2 + b2                       [S, S]
    q      = query @ wq + bq ; k = key @ wk + bk
    energy = q @ k^T                                               [S, S]
    attn   = softmax(dense + energy, axis=-1)
    out    = attn @ value
    return (out, attn)

Strategy: data-parallel over B=16 across 8 cores (2 batches/core).
Host-side algebra: softmax is shift-invariant per row, so
    q @ k^T  ==  query @ (wq wk^T) @ key^T + 1 (key @ wk bq)^T   (mod row consts)
Host precomputes W = wq@wk^T and kr = key @ (wk@bq); per-row-constant terms
(query@wq@bk and bq.bk) are dropped. Query/key are host-transposed to [D, S]
so every on-device matmul consumes natural layouts. Heavy matmuls run in
float32r (TF32) at full PE rate; the dense-synthesizer branch (augmented with
the b2 and kr rows) and attn@V run in bf16; softmax is fp32.
"""
import numpy as np
import ml_dtypes

import concourse.bass as bass
import concourse.mybir as mybir
import concourse.tile as tile
from concourse import bacc
from concourse.bass_utils import run_bass_kernel_spmd
from concourse.masks import make_identity

# problem sizes (hardcoded per spec)
B = 16
S = 2048
D = 1024
H = 64
NCORES = 8
BLOC = B // NCORES          # batches per core

P = 128
SLAB = 512                  # s-slab width for g/hidden production
NSLAB = S // SLAB           # 4
NT4 = 4                     # t-slabs of 512
TS = S // NT4               # 512
DC = D // P                 # 8 contraction chunks
HA = H + 2                  # hidden rows + b2-ones row + kr-ones row

f32 = mybir.dt.float32
f32r = mybir.dt.float32r
bf16 = mybir.dt.bfloat16

LAST_RESULT = None
_NC_CACHE = None


def _build_nc():
    nc = bacc.Bacc(None, target_bir_lowering=False)

    qt_d = nc.declare_dram_parameter("qt", [BLOC, D, S], f32r, isOutput=False)
    kt_d = nc.declare_dram_parameter("kt", [BLOC, D, S], f32r, isOutput=False)
    v_d = nc.declare_dram_parameter("v", [BLOC, S, D], bf16, isOutput=False)
    w_d = nc.declare_dram_parameter("w", [D, D], f32r, isOutput=False)
    w1_d = nc.declare_dram_parameter("w1", [D, H], f32r, isOutput=False)
    b1_d = nc.declare_dram_parameter("b1", [H, 1], f32, isOutput=False)
    w2b_d = nc.declare_dram_parameter("w2b", [H + 1, S], bf16, isOutput=False)
    kr_d = nc.declare_dram_parameter("kr", [BLOC, 1, S], bf16, isOutput=False)
    out_d = nc.declare_dram_parameter("out", [BLOC, S, D], f32, isOutput=True)
    attn_d = nc.declare_dram_parameter("attn", [BLOC, S, S], f32, isOutput=True)

    with tile.TileContext(nc) as tc:
        with (
            tc.tile_pool(name="const", bufs=1) as cpool,
            tc.tile_pool(name="batch", bufs=1) as bpool,
            tc.tile_pool(name="slab", bufs=1) as spool,
            tc.tile_pool(name="work", bufs=2) as wpool,
            tc.tile_pool(name="ps_sc", bufs=3, space="PSUM") as ps_sc,
            tc.tile_pool(name="ps_av", bufs=1, space="PSUM") as ps_av,
            tc.tile_pool(name="ps_g", bufs=1, space="PSUM") as ps_g,
            tc.tile_pool(name="ps_tp", bufs=2, space="PSUM") as ps_tp,
        ):
            # ---- constants (loaded once) ----
            ident_bf = cpool.tile([P, P], bf16)
            make_identity(nc, ident_bf[:])
            w_sb = cpool.tile([P, DC, D], f32r)       # W[d, :] chunks; 32KB/part
            for d in range(DC):
                nc.sync.dma_start(w_sb[:, d], w_d[d * P:(d + 1) * P, :])
            w1_sb = cpool.tile([P, DC, H], f32r)
            for d in range(DC):
                nc.sync.dma_start(w1_sb[:, d], w1_d[d * P:(d + 1) * P, :])
            b1_sb = cpool.tile([H, 1], f32)
            nc.sync.dma_start(b1_sb[:], b1_d[:])

            for b in range(BLOC):
                # ---- per-batch tensors ----
                w2b_sb = bpool.tile([HA, S], bf16, tag="w2b")
                nc.sync.dma_start(w2b_sb[0:H + 1, :], w2b_d[:])
                nc.sync.dma_start(w2b_sb[H + 1:HA, :], kr_d[b])
                kt_sb = bpool.tile([P, DC, S], f32r, tag="kt")      # 64KB/part
                for e in range(DC):
                    nc.sync.dma_start(kt_sb[:, e], kt_d[b][e * P:(e + 1) * P, :])
                v_sb = bpool.tile([P, S // P, D], bf16, tag="v")    # 32KB/part
                for t in range(S // P):
                    nc.sync.dma_start(v_sb[:, t], v_d[b][t * P:(t + 1) * P, :])
                ht_sb = bpool.tile([HA, S], bf16, tag="ht")
                nc.vector.memset(ht_sb[H:HA, :], 1.0)   # ones rows for b2 + kr

                for sl in range(NSLAB):
                    s_lo = sl * SLAB
                    qt_sb = spool.tile([P, DC, SLAB], f32r, tag="qt")
                    for d in range(DC):
                        nc.sync.dma_start(
                            qt_sb[:, d],
                            qt_d[b][d * P:(d + 1) * P, s_lo:s_lo + SLAB],
                        )
                    # ---- gT slab: g[e*128:(e+1)*128, s_slab] ----
                    g_sb = spool.tile([P, DC, SLAB], f32r, tag="g", bufs=1)
                    for e in range(DC):
                        pg = ps_g.tile([P, SLAB], f32, tag="pg")
                        for d in range(DC):
                            nc.tensor.matmul(
                                pg[:],
                                w_sb[:, d, e * P:(e + 1) * P],
                                qt_sb[:, d],
                                start=(d == 0),
                                stop=(d == DC - 1),
                            )
                        nc.scalar.copy(g_sb[:, e], pg[:])
                    # ---- hiddenT slab: relu(w1^T qT + b1) ----
                    ph_full = ps_g.tile([P, SLAB], f32, tag="pg")
                    ph = ph_full[0:H]
                    for d in range(DC):
                        nc.tensor.matmul(
                            ph,
                            w1_sb[:, d],
                            qt_sb[:, d],
                            start=(d == 0),
                            stop=(d == DC - 1),
                        )
                    nc.scalar.activation(
                        ht_sb[0:H, s_lo:s_lo + SLAB], ph,
                        mybir.ActivationFunctionType.Relu, bias=b1_sb[:],
                    )

                    for mt in range(SLAB // P):
                        s0 = s_lo + mt * P
                        scores = wpool.tile([P, S], f32, tag="scores")
                        for j in range(NT4):
                            t_lo = j * TS
                            psc = ps_sc.tile([P, TS], f32, tag="psc")
                            for e in range(DC):
                                nc.tensor.matmul(
                                    psc[:],
                                    g_sb[:, e, mt * P:(mt + 1) * P],
                                    kt_sb[:, e, t_lo:t_lo + TS],
                                    start=(e == 0),
                                    stop=False,
                                )
                            nc.tensor.matmul(
                                psc[:], ht_sb[:, s0:s0 + P], w2b_sb[:, t_lo:t_lo + TS],
                                start=False, stop=True,
                            )
                            nc.scalar.copy(scores[:, t_lo:t_lo + TS], psc[:])
                        # ---- softmax over the full row ----
                        # E_bf16 = exp(x - max) feeds the PE transposes at once;
                        # the f32 DRAM copy folds 1/sum via bias = -max - ln(sum);
                        # attn@v output is scaled by 1/sum on its PSUM copy.
                        negmax = wpool.tile([P, 1], f32, tag="negmax")
                        nc.vector.tensor_reduce(
                            negmax[:], scores[:], axis=mybir.AxisListType.X,
                            op=mybir.AluOpType.max, negate=True,
                        )
                        rowsum = wpool.tile([P, 1], f32, tag="rowsum")
                        attn_bf = wpool.tile([P, S], bf16, tag="attn_bf", bufs=1)
                        nc.scalar.activation(
                            attn_bf[:], scores[:], mybir.ActivationFunctionType.Exp,
                            bias=negmax[:], scale=1.0, accum_out=rowsum[:],
                        )
                        lnsum = wpool.tile([P, 1], f32, tag="lnsum")
                        nc.scalar.activation(
                            lnsum[:], rowsum[:], mybir.ActivationFunctionType.Ln,
                        )
                        negmax2 = wpool.tile([P, 1], f32, tag="negmax2")
                        nc.vector.tensor_tensor(
                            negmax2[:], negmax[:], lnsum[:], mybir.AluOpType.subtract,
                        )
                        recip = wpool.tile([P, 1], f32, tag="recip")
                        nc.vector.reciprocal(recip[:], rowsum[:])
                        # ---- attn^T (bf16) via PE transpose (unnormalized E) ----
                        at_bf = wpool.tile([P, S // P, P], bf16, tag="at", bufs=1)
                        for t in range(S // P):
                            tp = ps_tp.tile([P, P], bf16, tag="tp")
                            nc.tensor.transpose(
                                tp[:], attn_bf[:, t * P:(t + 1) * P], ident_bf[:]
                            )
                            nc.any.tensor_copy(at_bf[:, t], tp[:])
                        # ---- normalized f32 attention to DRAM ----
                        nc.scalar.activation(
                            scores[:], scores[:], mybir.ActivationFunctionType.Exp,
                            bias=negmax2[:], scale=1.0,
                        )
                        nc.sync.dma_start(attn_d[b][s0:s0 + P, :], scores[:])
                        # ---- out tile: attn @ v ----
                        po = ps_av.tile([P, D], f32, tag="po")
                        for t in range(S // P):
                            for ds in range(D // TS):
                                nc.tensor.matmul(
                                    po[:, ds * TS:(ds + 1) * TS],
                                    at_bf[:, t],
                                    v_sb[:, t, ds * TS:(ds + 1) * TS],
                                    start=(t == 0),
                                    stop=(t == S // P - 1),
                                )
                        o_sb = wpool.tile([P, D], f32, tag="o", bufs=1)
                        nc.scalar.mul(o_sb[:], po[:], recip[:])
                        nc.sync.dma_start(out_d[b][s0:s0 + P, :], o_sb[:])
    nc.compile()
    return nc


def _get_nc():
    global _NC_CACHE
    if _NC_CACHE is None:
        _NC_CACHE = _build_nc()
    return _NC_CACHE


def kernel(**inputs):
    global LAST_RESULT
    query = np.asarray(inputs["query"], dtype=np.float32)
    key = np.asarray(inputs["key"], dtype=np.float32)
    value = np.asarray(inputs["value"], dtype=np.float32)
    w1 = np.asarray(inputs["w1"], dtype=np.float32)
    b1 = np.asarray(inputs["b1"], dtype=np.float32)
    w2 = np.asarray(inputs["w2"], dtype=np.float32)
    b2 = np.asarray(inputs["b2"], dtype=np.float32)
    wq = np.asarray(inputs["wq"], dtype=np.float32)
    bq = np.asarray(inputs["bq"], dtype=np.float32)
    wk = np.asarray(inputs["wk"], dtype=np.float32)

    # host-side algebra (f64 for weight products)
    W = (wq.astype(np.float64) @ wk.astype(np.float64).T).astype(np.float32)
    r = wk.astype(np.float64) @ bq.astype(np.float64)          # [D]
    kr = (key.astype(np.float64) @ r).astype(np.float32)       # [B, S]

    qt = np.ascontiguousarray(query.transpose(0, 2, 1))        # [B, D, S]
    kt = np.ascontiguousarray(key.transpose(0, 2, 1))          # [B, D, S]
    v_bf = value.astype(ml_dtypes.bfloat16)
    w2b = np.concatenate([w2, b2[None, :]], axis=0).astype(ml_dtypes.bfloat16)
    kr_bf = kr.astype(ml_dtypes.bfloat16)
    b1c = np.ascontiguousarray(b1.reshape(H, 1))

    nc = _get_nc()
    in_maps = []
    for c in range(NCORES):
        lo = c * BLOC
        in_maps.append({
            "qt": qt[lo:lo + BLOC],
            "kt": kt[lo:lo + BLOC],
            "v": v_bf[lo:lo + BLOC],
            "w": W,
            "w1": w1,
            "b1": b1c,
            "w2b": w2b,
            "kr": kr_bf[lo:lo + BLOC].reshape(BLOC, 1, S),
        })

    res = run_bass_kernel_spmd(nc, in_maps, core_ids=list(range(NCORES)))
    LAST_RESULT = res
    out = np.concatenate([res.results[c]["out"] for c in range(NCORES)], axis=0)
    attn = np.concatenate([res.results[c]["attn"] for c in range(NCORES)], axis=0)
    return out, attn


# revision 10
# speedup vs baseline: 1.0350x; 1.0350x over previous
"""Dense+Vanilla Mixture Synthesizer attention kernel for 8 Trainium2 NeuronCores.

Reference computation (per batch b):
    dense  = relu(query @ w1 + b1) @ w2 + b2                       [S, S]
    q      = query @ wq + bq ; k = key @ wk + bk
    energy = q @ k^T                                               [S, S]
    attn   = softmax(dense + energy, axis=-1)
    out    = attn @ value
    return (out, attn)

Strategy: data-parallel over B=16 across 8 cores (2 batches/core).
Host-side algebra: softmax is shift-invariant per row, so
    q @ k^T  ==  query @ (wq wk^T) @ key^T + 1 (key @ wk bq)^T   (mod row consts)
Host precomputes W = wq@wk^T and kr = key @ (wk@bq); per-row-constant terms
(query@wq@bk and bq.bk) are dropped. Query/key are host-transposed to [D, S]
so every on-device matmul consumes natural layouts. Heavy matmuls run in
float32r (TF32) at full PE rate; the dense-synthesizer branch (augmented with
the b2 and kr rows) and attn@V run in bf16; softmax is fp32.
"""
import numpy as np
import ml_dtypes

import concourse.bass as bass
import concourse.mybir as mybir
import concourse.tile as tile
from concourse import bacc
from concourse.bass_utils import run_bass_kernel_spmd
from concourse.masks import make_identity

# problem sizes (hardcoded per spec)
B = 16
S = 2048
D = 1024
H = 64
NCORES = 8
BLOC = B // NCORES          # batches per core

P = 128
SLAB = 512                  # s-slab width for g/hidden production
NSLAB = S // SLAB           # 4
NT4 = 4                     # t-slabs of 512
TS = S // NT4               # 512
DC = D // P                 # 8 contraction chunks
HA = H + 2                  # hidden rows + b2-ones row + kr-ones row

f32 = mybir.dt.float32
f32r = mybir.dt.float32r
bf16 = mybir.dt.bfloat16

LAST_RESULT = None
_NC_CACHE = None


def _build_nc():
    nc = bacc.Bacc(None, target_bir_lowering=False)

    qt_d = nc.declare_dram_parameter("qt", [BLOC, D, S], f32r, isOutput=False)
    kt_d = nc.declare_dram_parameter("kt", [BLOC, D, S], f32r, isOutput=False)
    v_d = nc.declare_dram_parameter("v", [BLOC, S, D], bf16, isOutput=False)
    w_d = nc.declare_dram_parameter("w", [D, D], f32r, isOutput=False)
    w1_d = nc.declare_dram_parameter("w1", [D, H], f32r, isOutput=False)
    b1_d = nc.declare_dram_parameter("b1", [H, 1], f32, isOutput=False)
    w2b_d = nc.declare_dram_parameter("w2b", [H + 1, S], bf16, isOutput=False)
    kr_d = nc.declare_dram_parameter("kr", [BLOC, 1, S], bf16, isOutput=False)
    out_d = nc.declare_dram_parameter("out", [BLOC, S, D], f32, isOutput=True)
    attn_d = nc.declare_dram_parameter("attn", [BLOC, S, S], f32, isOutput=True)

    with tile.TileContext(nc) as tc:
        with (
            tc.tile_pool(name="const", bufs=1) as cpool,
            tc.tile_pool(name="batch", bufs=1) as bpool,
            tc.tile_pool(name="slab", bufs=1) as spool,
            tc.tile_pool(name="work", bufs=2) as wpool,
            tc.tile_pool(name="ps_sc", bufs=3, space="PSUM") as ps_sc,
            tc.tile_pool(name="ps_av", bufs=1, space="PSUM") as ps_av,
            tc.tile_pool(name="ps_g", bufs=1, space="PSUM") as ps_g,
            tc.tile_pool(name="ps_tp", bufs=2, space="PSUM") as ps_tp,
        ):
            # ---- constants (loaded once) ----
            ident_bf = cpool.tile([P, P], bf16)
            make_identity(nc, ident_bf[:])
            w_sb = cpool.tile([P, DC, D], f32r)       # W[d, :] chunks; 32KB/part
            for d in range(DC):
                nc.sync.dma_start(w_sb[:, d], w_d[d * P:(d + 1) * P, :])
            w1_sb = cpool.tile([P, DC, H], f32r)
            for d in range(DC):
                nc.sync.dma_start(w1_sb[:, d], w1_d[d * P:(d + 1) * P, :])
            b1_sb = cpool.tile([H, 1], f32)
            nc.sync.dma_start(b1_sb[:], b1_d[:])

            for b in range(BLOC):
                # ---- per-batch tensors ----
                w2b_sb = bpool.tile([HA, S], bf16, tag="w2b")
                nc.sync.dma_start(w2b_sb[0:H + 1, :], w2b_d[:])
                nc.sync.dma_start(w2b_sb[H + 1:HA, :], kr_d[b])
                kt_sb = bpool.tile([P, DC, S], f32r, tag="kt")      # 64KB/part
                for e in range(DC):
                    nc.sync.dma_start(kt_sb[:, e], kt_d[b][e * P:(e + 1) * P, :])
                v_sb = bpool.tile([P, S // P, D], bf16, tag="v")    # 32KB/part
                for t in range(S // P):
                    nc.sync.dma_start(v_sb[:, t], v_d[b][t * P:(t + 1) * P, :])
                ht_sb = bpool.tile([HA, S], bf16, tag="ht")
                nc.vector.memset(ht_sb[H:HA, :], 1.0)   # ones rows for b2 + kr

                for sl in range(NSLAB):
                    s_lo = sl * SLAB
                    qt_sb = spool.tile([P, DC, SLAB], f32r, tag="qt")
                    for d in range(DC):
                        nc.scalar.dma_start(
                            qt_sb[:, d],
                            qt_d[b][d * P:(d + 1) * P, s_lo:s_lo + SLAB],
                        )
                    # ---- gT slab: g[e*128:(e+1)*128, s_slab] ----
                    g_sb = spool.tile([P, DC, SLAB], f32r, tag="g", bufs=1)
                    for e in range(DC):
                        pg = ps_g.tile([P, SLAB], f32, tag="pg")
                        for d in range(DC):
                            nc.tensor.matmul(
                                pg[:],
                                w_sb[:, d, e * P:(e + 1) * P],
                                qt_sb[:, d],
                                start=(d == 0),
                                stop=(d == DC - 1),
                            )
                        nc.scalar.copy(g_sb[:, e], pg[:])
                    # ---- hiddenT slab: relu(w1^T qT + b1) ----
                    ph_full = ps_g.tile([P, SLAB], f32, tag="pg")
                    ph = ph_full[0:H]
                    for d in range(DC):
                        nc.tensor.matmul(
                            ph,
                            w1_sb[:, d],
                            qt_sb[:, d],
                            start=(d == 0),
                            stop=(d == DC - 1),
                        )
                    nc.scalar.activation(
                        ht_sb[0:H, s_lo:s_lo + SLAB], ph,
                        mybir.ActivationFunctionType.Relu, bias=b1_sb[:],
                    )

                    for mt in range(SLAB // P):
                        s0 = s_lo + mt * P
                        scores = wpool.tile([P, S], f32, tag="scores")
                        for j in range(NT4):
                            t_lo = j * TS
                            psc = ps_sc.tile([P, TS], f32, tag="psc")
                            for e in range(DC):
                                nc.tensor.matmul(
                                    psc[:],
                                    g_sb[:, e, mt * P:(mt + 1) * P],
                                    kt_sb[:, e, t_lo:t_lo + TS],
                                    start=(e == 0),
                                    stop=False,
                                )
                            nc.tensor.matmul(
                                psc[:], ht_sb[:, s0:s0 + P], w2b_sb[:, t_lo:t_lo + TS],
                                start=False, stop=True,
                            )
                            nc.scalar.copy(scores[:, t_lo:t_lo + TS], psc[:])
                        # ---- softmax over the full row ----
                        # E_bf16 = exp(x - max) feeds the PE transposes at once;
                        # the f32 DRAM copy folds 1/sum via bias = -max - ln(sum);
                        # attn@v output is scaled by 1/sum on its PSUM copy.
                        negmax = wpool.tile([P, 1], f32, tag="negmax")
                        nc.vector.tensor_reduce(
                            negmax[:], scores[:], axis=mybir.AxisListType.X,
                            op=mybir.AluOpType.max, negate=True,
                        )
                        rowsum = wpool.tile([P, 1], f32, tag="rowsum")
                        attn_bf = wpool.tile([P, S], bf16, tag="attn_bf", bufs=1)
                        nc.scalar.activation(
                            attn_bf[:], scores[:], mybir.ActivationFunctionType.Exp,
                            bias=negmax[:], scale=1.0, accum_out=rowsum[:],
                        )
                        lnsum = wpool.tile([P, 1], f32, tag="lnsum")
                        nc.scalar.activation(
                            lnsum[:], rowsum[:], mybir.ActivationFunctionType.Ln,
                        )
                        negmax2 = wpool.tile([P, 1], f32, tag="negmax2")
                        nc.vector.tensor_tensor(
                            negmax2[:], negmax[:], lnsum[:], mybir.AluOpType.subtract,
                        )
                        recip = wpool.tile([P, 1], f32, tag="recip")
                        nc.vector.reciprocal(recip[:], rowsum[:])
                        # ---- attn^T (bf16) via PE transpose (unnormalized E) ----
                        at_bf = wpool.tile([P, S // P, P], bf16, tag="at", bufs=1)
                        for t in range(S // P):
                            tp = ps_tp.tile([P, P], bf16, tag="tp")
                            nc.tensor.transpose(
                                tp[:], attn_bf[:, t * P:(t + 1) * P], ident_bf[:]
                            )
                            nc.any.tensor_copy(at_bf[:, t], tp[:])
                        # ---- normalized f32 attention to DRAM ----
                        nc.scalar.activation(
                            scores[:], scores[:], mybir.ActivationFunctionType.Exp,
                            bias=negmax2[:], scale=1.0,
                        )
                        nc.sync.dma_start(attn_d[b][s0:s0 + P, :], scores[:])
                        # ---- out tile: attn @ v ----
                        po = ps_av.tile([P, D], f32, tag="po")
                        for t in range(S // P):
                            for ds in range(D // TS):
                                nc.tensor.matmul(
                                    po[:, ds * TS:(ds + 1) * TS],
                                    at_bf[:, t],
                                    v_sb[:, t, ds * TS:(ds + 1) * TS],
                                    start=(t == 0),
                                    stop=(t == S // P - 1),
                                )
                        o_sb = wpool.tile([P, D], f32, tag="o", bufs=1)
                        nc.scalar.mul(o_sb[:], po[:], recip[:])
                        nc.sync.dma_start(out_d[b][s0:s0 + P, :], o_sb[:])
    nc.compile()
    return nc


def _get_nc():
    global _NC_CACHE
    if _NC_CACHE is None:
        _NC_CACHE = _build_nc()
    return _NC_CACHE


def kernel(**inputs):
    global LAST_RESULT
    query = np.asarray(inputs["query"], dtype=np.float32)
    key = np.asarray(inputs["key"], dtype=np.float32)
    value = np.asarray(inputs["value"], dtype=np.float32)
    w1 = np.asarray(inputs["w1"], dtype=np.float32)
    b1 = np.asarray(inputs["b1"], dtype=np.float32)
    w2 = np.asarray(inputs["w2"], dtype=np.float32)
    b2 = np.asarray(inputs["b2"], dtype=np.float32)
    wq = np.asarray(inputs["wq"], dtype=np.float32)
    bq = np.asarray(inputs["bq"], dtype=np.float32)
    wk = np.asarray(inputs["wk"], dtype=np.float32)

    # host-side algebra (f64 for weight products)
    W = (wq.astype(np.float64) @ wk.astype(np.float64).T).astype(np.float32)
    r = wk.astype(np.float64) @ bq.astype(np.float64)          # [D]
    kr = (key.astype(np.float64) @ r).astype(np.float32)       # [B, S]

    qt = np.ascontiguousarray(query.transpose(0, 2, 1))        # [B, D, S]
    kt = np.ascontiguousarray(key.transpose(0, 2, 1))          # [B, D, S]
    v_bf = value.astype(ml_dtypes.bfloat16)
    w2b = np.concatenate([w2, b2[None, :]], axis=0).astype(ml_dtypes.bfloat16)
    kr_bf = kr.astype(ml_dtypes.bfloat16)
    b1c = np.ascontiguousarray(b1.reshape(H, 1))

    nc = _get_nc()
    in_maps = []
    for c in range(NCORES):
        lo = c * BLOC
        in_maps.append({
            "qt": qt[lo:lo + BLOC],
            "kt": kt[lo:lo + BLOC],
            "v": v_bf[lo:lo + BLOC],
            "w": W,
            "w1": w1,
            "b1": b1c,
            "w2b": w2b,
            "kr": kr_bf[lo:lo + BLOC].reshape(BLOC, 1, S),
        })

    res = run_bass_kernel_spmd(nc, in_maps, core_ids=list(range(NCORES)))
    LAST_RESULT = res
    out = np.concatenate([res.results[c]["out"] for c in range(NCORES)], axis=0)
    attn = np.concatenate([res.results[c]["attn"] for c in range(NCORES)], axis=0)
    return out, attn


# revision 13
# speedup vs baseline: 1.0371x; 1.0020x over previous
"""Dense+Vanilla Mixture Synthesizer attention kernel for 8 Trainium2 NeuronCores.

Reference computation (per batch b):
    dense  = relu(query @ w1 + b1) @ w2 + b2                       [S, S]
    q      = query @ wq + bq ; k = key @ wk + bk
    energy = q @ k^T                                               [S, S]
    attn   = softmax(dense + energy, axis=-1)
    out    = attn @ value
    return (out, attn)

Strategy: data-parallel over B=16 across 8 cores (2 batches/core).
Host-side algebra: softmax is shift-invariant per row, so
    q @ k^T  ==  query @ (wq wk^T) @ key^T + 1 (key @ wk bq)^T   (mod row consts)
Host precomputes W = wq@wk^T and kr = key @ (wk@bq); per-row-constant terms
(query@wq@bk and bq.bk) are dropped. Query/key are host-transposed to [D, S]
so every on-device matmul consumes natural layouts. Heavy matmuls run in
float32r (TF32) at full PE rate; the dense-synthesizer branch (augmented with
the b2 and kr rows) and attn@V run in bf16; softmax is fp32.
"""
import numpy as np
import ml_dtypes

import concourse.bass as bass
import concourse.mybir as mybir
import concourse.tile as tile
from concourse import bacc
from concourse.bass_utils import run_bass_kernel_spmd
from concourse.masks import make_identity

# problem sizes (hardcoded per spec)
B = 16
S = 2048
D = 1024
H = 64
NCORES = 8
BLOC = B // NCORES          # batches per core

P = 128
SLAB = 512                  # s-slab width for g/hidden production
NSLAB = S // SLAB           # 4
NT4 = 4                     # t-slabs of 512
TS = S // NT4               # 512
DC = D // P                 # 8 contraction chunks
HA = H + 2                  # hidden rows + b2-ones row + kr-ones row

f32 = mybir.dt.float32
f32r = mybir.dt.float32r
bf16 = mybir.dt.bfloat16

LAST_RESULT = None
_NC_CACHE = None


def _build_nc():
    nc = bacc.Bacc(None, target_bir_lowering=False)

    qt_d = nc.declare_dram_parameter("qt", [BLOC, D, S], f32r, isOutput=False)
    kt_d = nc.declare_dram_parameter("kt", [BLOC, D, S], f32r, isOutput=False)
    v_d = nc.declare_dram_parameter("v", [BLOC, S, D], bf16, isOutput=False)
    w_d = nc.declare_dram_parameter("w", [D, D], f32r, isOutput=False)
    w1_d = nc.declare_dram_parameter("w1", [D, H], f32r, isOutput=False)
    b1_d = nc.declare_dram_parameter("b1", [H, 1], f32, isOutput=False)
    w2b_d = nc.declare_dram_parameter("w2b", [H + 1, S], bf16, isOutput=False)
    kr_d = nc.declare_dram_parameter("kr", [BLOC, 1, S], bf16, isOutput=False)
    out_d = nc.declare_dram_parameter("out", [BLOC, S, D], f32, isOutput=True)
    attn_d = nc.declare_dram_parameter("attn", [BLOC, S, S], f32, isOutput=True)

    with tile.TileContext(nc) as tc:
        with (
            tc.tile_pool(name="const", bufs=1) as cpool,
            tc.tile_pool(name="batch", bufs=1) as bpool,
            tc.tile_pool(name="slab", bufs=1) as spool,
            tc.tile_pool(name="work", bufs=2) as wpool,
            tc.tile_pool(name="ps_sc", bufs=3, space="PSUM") as ps_sc,
            tc.tile_pool(name="ps_av", bufs=1, space="PSUM") as ps_av,
            tc.tile_pool(name="ps_g", bufs=1, space="PSUM") as ps_g,
            tc.tile_pool(name="ps_tp", bufs=2, space="PSUM") as ps_tp,
        ):
            # ---- constants (loaded once) ----
            ident_bf = cpool.tile([P, P], bf16)
            make_identity(nc, ident_bf[:])
            w_sb = cpool.tile([P, DC, D], f32r)       # W[d, :] chunks; 32KB/part
            for d in range(DC):
                nc.sync.dma_start(w_sb[:, d], w_d[d * P:(d + 1) * P, :])
            w1_sb = cpool.tile([P, DC, H], f32r)
            for d in range(DC):
                nc.sync.dma_start(w1_sb[:, d], w1_d[d * P:(d + 1) * P, :])
            b1_sb = cpool.tile([H, 1], f32)
            nc.sync.dma_start(b1_sb[:], b1_d[:])

            for b in range(BLOC):
                # ---- per-batch tensors ----
                w2b_sb = bpool.tile([HA, S], bf16, tag="w2b")
                nc.sync.dma_start(w2b_sb[0:H + 1, :], w2b_d[:])
                nc.sync.dma_start(w2b_sb[H + 1:HA, :], kr_d[b])
                kt_sb = bpool.tile([P, DC, S], f32r, tag="kt")      # 64KB/part
                for e in range(DC):
                    nc.sync.dma_start(kt_sb[:, e], kt_d[b][e * P:(e + 1) * P, :])
                v_sb = bpool.tile([P, S // P, D], bf16, tag="v")    # 32KB/part
                for t in range(S // P):
                    nc.sync.dma_start(v_sb[:, t], v_d[b][t * P:(t + 1) * P, :])
                ht_sb = bpool.tile([HA, S], bf16, tag="ht")
                nc.vector.memset(ht_sb[H:HA, :], 1.0)   # ones rows for b2 + kr

                for sl in range(NSLAB):
                    s_lo = sl * SLAB
                    qt_sb = spool.tile([P, DC, SLAB], f32r, tag="qt")
                    for d in range(DC):
                        nc.scalar.dma_start(
                            qt_sb[:, d],
                            qt_d[b][d * P:(d + 1) * P, s_lo:s_lo + SLAB],
                        )
                    # ---- gT slab: g[e*128:(e+1)*128, s_slab] ----
                    g_sb = spool.tile([P, DC, SLAB], f32r, tag="g", bufs=1)
                    for e in range(DC):
                        pg = ps_g.tile([P, SLAB], f32, tag="pg")
                        for d in range(DC):
                            nc.tensor.matmul(
                                pg[:],
                                w_sb[:, d, e * P:(e + 1) * P],
                                qt_sb[:, d],
                                start=(d == 0),
                                stop=(d == DC - 1),
                            )
                        nc.scalar.copy(g_sb[:, e], pg[:])
                    # ---- hiddenT slab: relu(w1^T qT + b1) ----
                    ph_full = ps_g.tile([P, SLAB], f32, tag="pg")
                    ph = ph_full[0:H]
                    for d in range(DC):
                        nc.tensor.matmul(
                            ph,
                            w1_sb[:, d],
                            qt_sb[:, d],
                            start=(d == 0),
                            stop=(d == DC - 1),
                        )
                    nc.scalar.activation(
                        ht_sb[0:H, s_lo:s_lo + SLAB], ph,
                        mybir.ActivationFunctionType.Relu, bias=b1_sb[:],
                    )

                    for mt in range(SLAB // P):
                        s0 = s_lo + mt * P
                        scores = wpool.tile([P, S], f32, tag="scores")
                        for j in range(NT4):
                            t_lo = j * TS
                            psc = ps_sc.tile([P, TS], f32, tag="psc")
                            for e in range(DC):
                                nc.tensor.matmul(
                                    psc[:],
                                    g_sb[:, e, mt * P:(mt + 1) * P],
                                    kt_sb[:, e, t_lo:t_lo + TS],
                                    start=(e == 0),
                                    stop=False,
                                )
                            nc.tensor.matmul(
                                psc[:], ht_sb[:, s0:s0 + P], w2b_sb[:, t_lo:t_lo + TS],
                                start=False, stop=True,
                            )
                            nc.scalar.copy(scores[:, t_lo:t_lo + TS], psc[:])
                        # ---- softmax over the full row ----
                        # E_bf16 = exp(x - max) feeds the PE transposes at once;
                        # the f32 DRAM copy folds 1/sum via bias = -max - ln(sum);
                        # attn@v output is scaled by 1/sum on its PSUM copy.
                        negmax = wpool.tile([P, 1], f32, tag="negmax")
                        nc.vector.tensor_reduce(
                            negmax[:], scores[:], axis=mybir.AxisListType.X,
                            op=mybir.AluOpType.max, negate=True,
                        )
                        rowsum = wpool.tile([P, 1], f32, tag="rowsum")
                        attn_bf = wpool.tile([P, S], bf16, tag="attn_bf", bufs=2)
                        nc.scalar.activation(
                            attn_bf[:], scores[:], mybir.ActivationFunctionType.Exp,
                            bias=negmax[:], scale=1.0, accum_out=rowsum[:],
                        )
                        lnsum = wpool.tile([P, 1], f32, tag="lnsum")
                        nc.scalar.activation(
                            lnsum[:], rowsum[:], mybir.ActivationFunctionType.Ln,
                        )
                        negmax2 = wpool.tile([P, 1], f32, tag="negmax2")
                        nc.vector.tensor_tensor(
                            negmax2[:], negmax[:], lnsum[:], mybir.AluOpType.subtract,
                        )
                        recip = wpool.tile([P, 1], f32, tag="recip")
                        nc.vector.reciprocal(recip[:], rowsum[:])
                        # ---- attn^T (bf16) via PE transpose (unnormalized E) ----
                        at_bf = wpool.tile([P, S // P, P], bf16, tag="at", bufs=1)
                        for t in range(S // P):
                            tp = ps_tp.tile([P, P], bf16, tag="tp")
                            nc.tensor.transpose(
                                tp[:], attn_bf[:, t * P:(t + 1) * P], ident_bf[:]
                            )
                            nc.any.tensor_copy(at_bf[:, t], tp[:])
                        # ---- normalized f32 attention to DRAM ----
                        nc.scalar.activation(
                            scores[:], scores[:], mybir.ActivationFunctionType.Exp,
                            bias=negmax2[:], scale=1.0,
                        )
                        nc.sync.dma_start(attn_d[b][s0:s0 + P, :], scores[:])
                        # ---- out tile: attn @ v ----
                        po = ps_av.tile([P, D], f32, tag="po")
                        for t in range(S // P):
                            for ds in range(D // TS):
                                nc.tensor.matmul(
                                    po[:, ds * TS:(ds + 1) * TS],
                                    at_bf[:, t],
                                    v_sb[:, t, ds * TS:(ds + 1) * TS],
                                    start=(t == 0),
                                    stop=(t == S // P - 1),
                                )
                        o_sb = wpool.tile([P, D], f32, tag="o", bufs=1)
                        nc.scalar.mul(o_sb[:], po[:], recip[:])
                        nc.sync.dma_start(out_d[b][s0:s0 + P, :], o_sb[:])
    nc.compile()
    return nc


def _get_nc():
    global _NC_CACHE
    if _NC_CACHE is None:
        _NC_CACHE = _build_nc()
    return _NC_CACHE


def kernel(**inputs):
    global LAST_RESULT
    query = np.asarray(inputs["query"], dtype=np.float32)
    key = np.asarray(inputs["key"], dtype=np.float32)
    value = np.asarray(inputs["value"], dtype=np.float32)
    w1 = np.asarray(inputs["w1"], dtype=np.float32)
    b1 = np.asarray(inputs["b1"], dtype=np.float32)
    w2 = np.asarray(inputs["w2"], dtype=np.float32)
    b2 = np.asarray(inputs["b2"], dtype=np.float32)
    wq = np.asarray(inputs["wq"], dtype=np.float32)
    bq = np.asarray(inputs["bq"], dtype=np.float32)
    wk = np.asarray(inputs["wk"], dtype=np.float32)

    # host-side algebra (f64 for weight products)
    W = (wq.astype(np.float64) @ wk.astype(np.float64).T).astype(np.float32)
    r = wk.astype(np.float64) @ bq.astype(np.float64)          # [D]
    kr = (key.astype(np.float64) @ r).astype(np.float32)       # [B, S]

    qt = np.ascontiguousarray(query.transpose(0, 2, 1))        # [B, D, S]
    kt = np.ascontiguousarray(key.transpose(0, 2, 1))          # [B, D, S]
    v_bf = value.astype(ml_dtypes.bfloat16)
    w2b = np.concatenate([w2, b2[None, :]], axis=0).astype(ml_dtypes.bfloat16)
    kr_bf = kr.astype(ml_dtypes.bfloat16)
    b1c = np.ascontiguousarray(b1.reshape(H, 1))

    nc = _get_nc()
    in_maps = []
    for c in range(NCORES):
        lo = c * BLOC
        in_maps.append({
            "qt": qt[lo:lo + BLOC],
            "kt": kt[lo:lo + BLOC],
            "v": v_bf[lo:lo + BLOC],
            "w": W,
            "w1": w1,
            "b1": b1c,
            "w2b": w2b,
            "kr": kr_bf[lo:lo + BLOC].reshape(BLOC, 1, S),
        })

    res = run_bass_kernel_spmd(nc, in_maps, core_ids=list(range(NCORES)))
    LAST_RESULT = res
    out = np.concatenate([res.results[c]["out"] for c in range(NCORES)], axis=0)
    attn = np.concatenate([res.results[c]["attn"] for c in range(NCORES)], axis=0)
    return out, attn


# revision 15
# speedup vs baseline: 1.0398x; 1.0026x over previous
"""Dense+Vanilla Mixture Synthesizer attention kernel for 8 Trainium2 NeuronCores.

Reference computation (per batch b):
    dense  = relu(query @ w1 + b1) @ w2 + b2                       [S, S]
    q      = query @ wq + bq ; k = key @ wk + bk
    energy = q @ k^T                                               [S, S]
    attn   = softmax(dense + energy, axis=-1)
    out    = attn @ value
    return (out, attn)

Strategy: data-parallel over B=16 across 8 cores (2 batches/core).
Host-side algebra: softmax is shift-invariant per row, so
    q @ k^T  ==  query @ (wq wk^T) @ key^T + 1 (key @ wk bq)^T   (mod row consts)
Host precomputes W = wq@wk^T and kr = key @ (wk@bq); per-row-constant terms
(query@wq@bk and bq.bk) are dropped. Query/key are host-transposed to [D, S]
so every on-device matmul consumes natural layouts. Heavy matmuls run in
float32r (TF32) at full PE rate; the dense-synthesizer branch (augmented with
the b2 and kr rows) and attn@V run in bf16; softmax is fp32.
"""
import numpy as np
import ml_dtypes

import concourse.bass as bass
import concourse.mybir as mybir
import concourse.tile as tile
from concourse import bacc
from concourse.bass_utils import run_bass_kernel_spmd
from concourse.masks import make_identity

# problem sizes (hardcoded per spec)
B = 16
S = 2048
D = 1024
H = 64
NCORES = 8
BLOC = B // NCORES          # batches per core

P = 128
SLAB = 512                  # s-slab width for g/hidden production
NSLAB = S // SLAB           # 4
NT4 = 4                     # t-slabs of 512
TS = S // NT4               # 512
DC = D // P                 # 8 contraction chunks
HA = H + 2                  # hidden rows + b2-ones row + kr-ones row

f32 = mybir.dt.float32
f32r = mybir.dt.float32r
bf16 = mybir.dt.bfloat16

LAST_RESULT = None
_NC_CACHE = None


def _build_nc():
    nc = bacc.Bacc(None, target_bir_lowering=False)

    qt_d = nc.declare_dram_parameter("qt", [BLOC, D, S], f32r, isOutput=False)
    kt_d = nc.declare_dram_parameter("kt", [BLOC, D, S], f32r, isOutput=False)
    v_d = nc.declare_dram_parameter("v", [BLOC, S, D], bf16, isOutput=False)
    w_d = nc.declare_dram_parameter("w", [D, D], f32r, isOutput=False)
    w1_d = nc.declare_dram_parameter("w1", [D, H], f32r, isOutput=False)
    b1_d = nc.declare_dram_parameter("b1", [H, 1], f32, isOutput=False)
    w2b_d = nc.declare_dram_parameter("w2b", [H + 1, S], bf16, isOutput=False)
    kr_d = nc.declare_dram_parameter("kr", [BLOC, 1, S], bf16, isOutput=False)
    out_d = nc.declare_dram_parameter("out", [BLOC, S, D], f32, isOutput=True)
    attn_d = nc.declare_dram_parameter("attn", [BLOC, S, S], f32, isOutput=True)

    with tile.TileContext(nc) as tc:
        with (
            tc.tile_pool(name="const", bufs=1) as cpool,
            tc.tile_pool(name="batch", bufs=1) as bpool,
            tc.tile_pool(name="slab", bufs=1) as spool,
            tc.tile_pool(name="work", bufs=2) as wpool,
            tc.tile_pool(name="ps_sc", bufs=3, space="PSUM") as ps_sc,
            tc.tile_pool(name="ps_av", bufs=1, space="PSUM") as ps_av,
            tc.tile_pool(name="ps_g", bufs=1, space="PSUM") as ps_g,
            tc.tile_pool(name="ps_tp", bufs=2, space="PSUM") as ps_tp,
        ):
            # ---- constants (loaded once) ----
            ident_bf = cpool.tile([P, P], bf16)
            make_identity(nc, ident_bf[:])
            w_sb = cpool.tile([P, DC, D], f32r)       # W[d, :] chunks; 32KB/part
            for d in range(DC):
                nc.sync.dma_start(w_sb[:, d], w_d[d * P:(d + 1) * P, :])
            w1_sb = cpool.tile([P, DC, H], f32r)
            for d in range(DC):
                nc.sync.dma_start(w1_sb[:, d], w1_d[d * P:(d + 1) * P, :])
            b1_sb = cpool.tile([H, 1], f32)
            nc.sync.dma_start(b1_sb[:], b1_d[:])

            for b in range(BLOC):
                # ---- per-batch tensors ----
                w2b_sb = bpool.tile([HA, S], bf16, tag="w2b")
                nc.sync.dma_start(w2b_sb[0:H + 1, :], w2b_d[:])
                nc.sync.dma_start(w2b_sb[H + 1:HA, :], kr_d[b])
                kt_sb = bpool.tile([P, DC, S], f32r, tag="kt")      # 64KB/part
                for e in range(DC):
                    nc.sync.dma_start(kt_sb[:, e], kt_d[b][e * P:(e + 1) * P, :])
                v_sb = bpool.tile([P, S // P, D], bf16, tag="v")    # 32KB/part
                for t in range(S // P):
                    nc.sync.dma_start(v_sb[:, t], v_d[b][t * P:(t + 1) * P, :])
                ht_sb = bpool.tile([HA, S], bf16, tag="ht")
                nc.vector.memset(ht_sb[H:HA, :], 1.0)   # ones rows for b2 + kr

                for sl in range(NSLAB):
                    s_lo = sl * SLAB
                    qt_sb = spool.tile([P, DC, SLAB], f32r, tag="qt")
                    for d in range(DC):
                        nc.scalar.dma_start(
                            qt_sb[:, d],
                            qt_d[b][d * P:(d + 1) * P, s_lo:s_lo + SLAB],
                        )
                    # ---- gT slab: g[e*128:(e+1)*128, s_slab] ----
                    g_sb = spool.tile([P, DC, SLAB], f32r, tag="g", bufs=1)
                    for e in range(DC):
                        pg = ps_g.tile([P, SLAB], f32, tag="pg")
                        for d in range(DC):
                            nc.tensor.matmul(
                                pg[:],
                                w_sb[:, d, e * P:(e + 1) * P],
                                qt_sb[:, d],
                                start=(d == 0),
                                stop=(d == DC - 1),
                            )
                        nc.scalar.copy(g_sb[:, e], pg[:])
                    # ---- hiddenT slab: relu(w1^T qT + b1) ----
                    ph_full = ps_g.tile([P, SLAB], f32, tag="pg")
                    ph = ph_full[0:H]
                    for d in range(DC):
                        nc.tensor.matmul(
                            ph,
                            w1_sb[:, d],
                            qt_sb[:, d],
                            start=(d == 0),
                            stop=(d == DC - 1),
                        )
                    nc.scalar.activation(
                        ht_sb[0:H, s_lo:s_lo + SLAB], ph,
                        mybir.ActivationFunctionType.Relu, bias=b1_sb[:],
                    )

                    for mt in range(SLAB // P):
                        s0 = s_lo + mt * P
                        scores = wpool.tile([P, S], f32, tag="scores", bufs=3)
                        for j in range(NT4):
                            t_lo = j * TS
                            psc = ps_sc.tile([P, TS], f32, tag="psc")
                            for e in range(DC):
                                nc.tensor.matmul(
                                    psc[:],
                                    g_sb[:, e, mt * P:(mt + 1) * P],
                                    kt_sb[:, e, t_lo:t_lo + TS],
                                    start=(e == 0),
                                    stop=False,
                                )
                            nc.tensor.matmul(
                                psc[:], ht_sb[:, s0:s0 + P], w2b_sb[:, t_lo:t_lo + TS],
                                start=False, stop=True,
                            )
                            nc.scalar.copy(scores[:, t_lo:t_lo + TS], psc[:])
                        # ---- softmax over the full row ----
                        # E_bf16 = exp(x - max) feeds the PE transposes at once;
                        # the f32 DRAM copy folds 1/sum via bias = -max - ln(sum);
                        # attn@v output is scaled by 1/sum on its PSUM copy.
                        negmax = wpool.tile([P, 1], f32, tag="negmax")
                        nc.vector.tensor_reduce(
                            negmax[:], scores[:], axis=mybir.AxisListType.X,
                            op=mybir.AluOpType.max, negate=True,
                        )
                        rowsum = wpool.tile([P, 1], f32, tag="rowsum")
                        attn_bf = wpool.tile([P, S], bf16, tag="attn_bf", bufs=1)
                        nc.scalar.activation(
                            attn_bf[:], scores[:], mybir.ActivationFunctionType.Exp,
                            bias=negmax[:], scale=1.0, accum_out=rowsum[:],
                        )
                        lnsum = wpool.tile([P, 1], f32, tag="lnsum")
                        nc.scalar.activation(
                            lnsum[:], rowsum[:], mybir.ActivationFunctionType.Ln,
                        )
                        negmax2 = wpool.tile([P, 1], f32, tag="negmax2")
                        nc.vector.tensor_tensor(
                            negmax2[:], negmax[:], lnsum[:], mybir.AluOpType.subtract,
                        )
                        recip = wpool.tile([P, 1], f32, tag="recip")
                        nc.vector.reciprocal(recip[:], rowsum[:])
                        # ---- attn^T (bf16) via PE transpose (unnormalized E) ----
                        at_bf = wpool.tile([P, S // P, P], bf16, tag="at", bufs=1)
                        for t in range(S // P):
                            tp = ps_tp.tile([P, P], bf16, tag="tp")
                            nc.tensor.transpose(
                                tp[:], attn_bf[:, t * P:(t + 1) * P], ident_bf[:]
                            )
                            nc.any.tensor_copy(at_bf[:, t], tp[:])
                        # ---- normalized f32 attention to DRAM ----
                        nc.scalar.activation(
                            scores[:], scores[:], mybir.ActivationFunctionType.Exp,
                            bias=negmax2[:], scale=1.0,
                        )
                        nc.sync.dma_start(attn_d[b][s0:s0 + P, :], scores[:])
                        # ---- out tile: attn @ v ----
                        o_sb = wpool.tile([P, D], f32, tag="o", bufs=1)
                        for ds in range(D // TS):
                            po = ps_av.tile([P, TS], f32, tag="po", bufs=2)
                            for t in range(S // P):
                                nc.tensor.matmul(
                                    po[:],
                                    at_bf[:, t],
                                    v_sb[:, t, ds * TS:(ds + 1) * TS],
                                    start=(t == 0),
                                    stop=(t == S // P - 1),
                                )
                            nc.scalar.mul(o_sb[:, ds * TS:(ds + 1) * TS], po[:], recip[:])
                        nc.sync.dma_start(out_d[b][s0:s0 + P, :], o_sb[:])
    nc.compile()
    return nc


def _get_nc():
    global _NC_CACHE
    if _NC_CACHE is None:
        _NC_CACHE = _build_nc()
    return _NC_CACHE


def kernel(**inputs):
    global LAST_RESULT
    query = np.asarray(inputs["query"], dtype=np.float32)
    key = np.asarray(inputs["key"], dtype=np.float32)
    value = np.asarray(inputs["value"], dtype=np.float32)
    w1 = np.asarray(inputs["w1"], dtype=np.float32)
    b1 = np.asarray(inputs["b1"], dtype=np.float32)
    w2 = np.asarray(inputs["w2"], dtype=np.float32)
    b2 = np.asarray(inputs["b2"], dtype=np.float32)
    wq = np.asarray(inputs["wq"], dtype=np.float32)
    bq = np.asarray(inputs["bq"], dtype=np.float32)
    wk = np.asarray(inputs["wk"], dtype=np.float32)

    # host-side algebra (f64 for weight products)
    W = (wq.astype(np.float64) @ wk.astype(np.float64).T).astype(np.float32)
    r = wk.astype(np.float64) @ bq.astype(np.float64)          # [D]
    kr = (key.astype(np.float64) @ r).astype(np.float32)       # [B, S]

    qt = np.ascontiguousarray(query.transpose(0, 2, 1))        # [B, D, S]
    kt = np.ascontiguousarray(key.transpose(0, 2, 1))          # [B, D, S]
    v_bf = value.astype(ml_dtypes.bfloat16)
    w2b = np.concatenate([w2, b2[None, :]], axis=0).astype(ml_dtypes.bfloat16)
    kr_bf = kr.astype(ml_dtypes.bfloat16)
    b1c = np.ascontiguousarray(b1.reshape(H, 1))

    nc = _get_nc()
    in_maps = []
    for c in range(NCORES):
        lo = c * BLOC
        in_maps.append({
            "qt": qt[lo:lo + BLOC],
            "kt": kt[lo:lo + BLOC],
            "v": v_bf[lo:lo + BLOC],
            "w": W,
            "w1": w1,
            "b1": b1c,
            "w2b": w2b,
            "kr": kr_bf[lo:lo + BLOC].reshape(BLOC, 1, S),
        })

    res = run_bass_kernel_spmd(nc, in_maps, core_ids=list(range(NCORES)))
    LAST_RESULT = res
    out = np.concatenate([res.results[c]["out"] for c in range(NCORES)], axis=0)
    attn = np.concatenate([res.results[c]["attn"] for c in range(NCORES)], axis=0)
    return out, attn
